# revision 27
# baseline (speedup 1.0000x reference)
"""Trainium2 Bass kernel for windowless 3D relative-position attention.

Full-input contract: kernel(**inputs) takes the unsharded numpy inputs and
returns the full [4, 2048, 256] output. Internally shards across 8 NeuronCores
as (batch b = core//2) x (head-group g = core%2, 4 heads each).

v3 design (ACT-engine-walled, everything overlapped under it):
  - expBT resident in SBUF ([128, 16, 2048] fp16, 64KB/partition) loaded once.
  - scores for head h row-packed at tile_position (32h, 0); sc tiles
    [128, 2, 512] double-buffered so ACT(exp) never stalls.
  - software pipeline: scores(m+1) issued before AV(m) so the tensor
    stream never blocks ACT.
  - AV accumulators for 4 heads packed into 2 PSUM banks (head pairs at
    PE column offsets 0 / 64 -> psum partitions 0-32 / 64-96).
  - softmax denominators staged psum -> SBUF -> DRAM -> [128,16] so the
    reciprocal runs on 128 partitions; recip broadcast back via DMA.
    All engine ops stay lane-aligned (equal partition bases).
  - quarter-q normalization work is interleaved into quarter q+1's stream
    so neither ACT nor DVE bubbles at quarter boundaries.
  - output projection: two K=128 matmuls per n-tile against zero-padded
    w_out halves (heads at partitions 0-31 / 64-95), ACT copies psum ->
    SBUF, DMA out.

The bias add is factored through the exponential: exp(s+bias) =
exp(s)*exp(bias), with exp(bias.T - C_SHIFT) precomputed on host in fp16
(C_SHIFT keeps products in fp16 range; it cancels in the softmax ratio).
"""

import os
import sys
from contextlib import ExitStack

import numpy as np

sys.path.insert(0, "/opt/trn_rl_repo")

import concourse.bass as bass
import concourse.bacc as bacc
import concourse.tile as tile
from concourse import mybir
from concourse.bass_utils import run_bass_kernel_spmd

# Problem constants (hardcoded per contract)
B = 4
N = 2048
INP = 256
OUP = 256
HEADS = 8
DIM_HEAD = 32
SCALE = DIM_HEAD ** -0.5
HL = 4            # heads per core
MT = N // 128     # 16 m-tiles (keys)
NQ = 4            # 512-wide n (query) quarters
NQW = 512
C_SHIFT = 4.0

f32 = mybir.dt.float32
f16 = mybir.dt.float16

_LAST = {"exec_time_ns": None}


def _build_nc():
    nc = bacc.Bacc("TRN2", target_bir_lowering=False, debug=False)
    xT_d = nc.dram_tensor("xT", [2, 128, N], f16, kind="ExternalInput")
    wqk_d = nc.dram_tensor("w_qk", [2, 128, 256], f16, kind="ExternalInput")
    wv_d = nc.dram_tensor("w_v", [2, 128, 128], f16, kind="ExternalInput")
    wout_d = nc.dram_tensor("w_out2", [128, 256], f16, kind="ExternalInput")
    ebt_d = nc.dram_tensor("expbt", [N, N], f16, kind="ExternalInput")
    out_d = nc.dram_tensor("partial", [N, OUP], f32, kind="ExternalOutput")
    den_d = nc.dram_tensor("den_scratch", [NQ, HL, NQW], f32)
    rec_d = nc.dram_tensor("rec_scratch", [NQ, HL, NQW], f32)

    with ExitStack() as ctx:
        tc = ctx.enter_context(tile.TileContext(nc))
        consts = ctx.enter_context(tc.tile_pool(name="consts", bufs=1))

        ebt = consts.tile([128, MT, N], f16)          # [m%128, mtile, n]
        xT = consts.tile([128, 2, N], f16)
        wqk = consts.tile([128, 2, 256], f16)
        wv = consts.tile([128, 2, 128], f16)
        woutd = consts.tile([128, 256], f16)
        qkT = consts.tile([128, 2, N], f16)           # [:,0,:]=qT  [:,1,:]=kT
        vsb = consts.tile([128, MT, HL, 33], f16)     # [m%128, mtile, head, d|ones]
        aoutT = consts.tile([128, N], f16)            # [(h,d), n] normalized
        den_q = consts.tile([128, 4, NQW], f32)       # partition 32, col=head
        den_sb = consts.tile([128, 16], f32)
        rec_sb = consts.tile([128, 16], f32)

        # first columns of x + w_qk first (unblocks projection chunk 0),
        # then the rest; the big ebt stream follows behind
        for kk in range(2):
            nc.sync.dma_start(out=xT[:, kk, 0:N // 2], in_=xT_d[kk, :, 0:N // 2])
        for kk in range(2):
            nc.sync.dma_start(out=wqk[:, kk, :], in_=wqk_d[kk])
        for kk in range(2):
            nc.sync.dma_start(out=xT[:, kk, N // 2:N], in_=xT_d[kk, :, N // 2:N])
        for kk in range(2):
            nc.sync.dma_start(out=wv[:, kk, :], in_=wv_d[kk])
        nc.sync.dma_start(out=woutd[:], in_=wout_d[:])
        for m in range(MT):
            nc.sync.dma_start(out=ebt[:, m, :], in_=ebt_d[m * 128:(m + 1) * 128, :])
        nc.vector.memset(vsb[:], 1.0)

        # --- q/k projection (transposed orientation) ---
        with tc.tile_pool(name="ppsq", bufs=8, space="PSUM") as ppsq:
            for ch in range(NQ):
                for mb in range(2):       # 0 -> q block, 1 -> k block
                    ps = ppsq.tile([128, 512], f32, tag="qkps")
                    for kk in range(2):
                        nc.tensor.matmul(
                            ps[:],
                            lhsT=wqk[:, kk, mb * 128:(mb + 1) * 128],
                            rhs=xT[:, kk, ch * 512:(ch + 1) * 512],
                            start=(kk == 0), stop=(kk == 1),
                        )
                    nc.vector.tensor_copy(
                        out=qkT[:, mb, ch * 512:(ch + 1) * 512], in_=ps[:]
                    )
        woutc = consts.tile([128, 256], f16)

        # --- attention ---
        # PSUM: sc pool (tags sc0/sc1, 1 buf, 2 banks each) on the freed
        # q/k-proj banks + v-proj pool (4 banks) which hands over to the
        # oa pool after the prologue. First scores/exp issue BEFORE the
        # v-projections so ACT starts ~15us earlier.
        with tc.tile_pool(name="awp", bufs=4) as awp, \
             tc.tile_pool(name="aw2p", bufs=4) as aw2p, \
             tc.tile_pool(name="rbp", bufs=8) as rbp:
          with tc.tile_pool(name="sps", bufs=1, space="PSUM") as sps:

            def issue_scores(m, ncol0):
                scs = []
                for hp in range(2):
                    sc = sps.tile([128, 2, NQW], f32, tag=f"sc{hp}")
                    for hi in range(2):
                        hl = hp * 2 + hi
                        nc.tensor.matmul(
                            sc[:, hi, :],
                            lhsT=qkT[32 * hl:32 * (hl + 1), 1,
                                     m * 128:(m + 1) * 128],
                            rhs=qkT[32 * hl:32 * (hl + 1), 0,
                                    ncol0:ncol0 + NQW],
                            start=True, stop=True,
                            tile_position=(32 * hl, 0),
                        )
                    scs.append(sc)
                return scs

            def issue_act_mul(m, ncol0, scs):
                aw2s = []
                for hp in range(2):
                    sc = scs[hp]
                    aw = awp.tile([128, 2, NQW], f16)
                    nc.scalar.activation(
                        out=aw[:], in_=sc[:],
                        func=mybir.ActivationFunctionType.Exp,
                        scale=SCALE,
                    )
                    ebs = ebt[:, m, ncol0:ncol0 + NQW]
                    eb_b = bass.AP(
                        tensor=ebs.tensor, offset=ebs.offset,
                        ap=[ebs.ap[0], [0, 2], ebs.ap[1]],
                    )
                    aw2 = aw2p.tile([128, 2, NQW], f16, tag="aw2")
                    nc.vector.tensor_mul(aw2[:], aw[:], eb_b)
                    aw2s.append(aw2)
                return aw2s

            def issue_av(m, aw2s, oa):
                for hp in range(2):
                    for hi in range(2):
                        hl = hp * 2 + hi
                        po = 64 * (hl % 2)
                        nc.tensor.matmul(
                            oa[hl // 2][po:po + 33, :],
                            lhsT=vsb[:, m, hl, :],
                            rhs=aw2s[hp][:, hi, :],
                            start=(m == 0), stop=(m == MT - 1),
                        )

            # prologue: first scores + exp/mul of quarter 0, THEN the
            # v-projection (tensor chews it while ACT runs)
            scs0 = issue_scores(0, 0)
            aw2s0 = issue_act_mul(0, 0, scs0)
            with tc.tile_pool(name="ppsv", bufs=4, space="PSUM") as ppsv:
                for nt in range(MT):
                    ps = ppsv.tile([128, 128], f32, tag="vps")
                    for kk in range(2):
                        nc.tensor.matmul(
                            ps[:],
                            lhsT=xT[:, kk, nt * 128:(nt + 1) * 128],
                            rhs=wv[:, kk, :],
                            start=(kk == 0), stop=(kk == 1),
                        )
                    nc.vector.tensor_copy(out=vsb[:, nt, :, 0:32], in_=ps[:])
            nc.vector.tensor_copy(out=woutc[:], in_=woutd[:])

            with tc.tile_pool(name="oap", bufs=2, space="PSUM") as oap:
                pending = []   # deferred normalize steps, previous quarter

                for q in range(NQ):
                    ncol0 = q * NQW
                    oa = [oap.tile([128, NQW], f32, tag=f"oa{i}",
                                   name=f"oa{i}_{q}")
                          for i in range(2)]

                    def make_norm_steps(qq, oa_q):
                        def stepA():
                            for i in range(2):
                                nc.vector.tensor_copy(
                                    out=den_q[32:33, 2 * i, :],
                                    in_=oa_q[i][32:33, :],
                                )
                                nc.vector.tensor_copy(
                                    out=den_q[32:33, 2 * i + 1, :],
                                    in_=oa_q[i][96:97, :],
                                )
                            nc.sync.dma_start(out=den_d[qq],
                                              in_=den_q[32:33, :, :])
                            dsl = den_d[qq]
                            den128 = bass.AP(
                                tensor=dsl.tensor, offset=dsl.offset,
                                ap=[[16, 128], [1, 16]],
                            )
                            nc.sync.dma_start(out=den_sb[:, :], in_=den128)

                        def stepB():
                            nc.vector.reciprocal(out=rec_sb[:, :],
                                                 in_=den_sb[:, :])
                            rsl = rec_d[qq]
                            rec128 = bass.AP(
                                tensor=rsl.tensor, offset=rsl.offset,
                                ap=[[16, 128], [1, 16]],
                            )
                            nc.sync.dma_start(out=rec128, in_=rec_sb[:, :])

                        def make_stepC(hl):
                            def stepC():
                                po = 64 * (hl % 2)
                                rb = rbp.tile([32, NQW], f32, tag=f"rb{hl}",
                                              name=f"rb{hl}_{qq}")
                                rsrc = rec_d[qq, hl]
                                rb_src = bass.AP(
                                    tensor=rsrc.tensor, offset=rsrc.offset,
                                    ap=[[0, 32], rsrc.ap[-1]],
                                )
                                nc.sync.dma_start(out=rb[0:32, :], in_=rb_src)
                                nc.vector.tensor_mul(
                                    aoutT[32 * hl:32 * hl + 32,
                                          qq * NQW:(qq + 1) * NQW],
                                    oa_q[hl // 2][po:po + 32, :],
                                    rb[0:32, :],
                                )
                            return stepC

                        return [stepA, stepB] + [make_stepC(hl)
                                                 for hl in range(HL)]

                    scs = scs0 if q == 0 else issue_scores(0, ncol0)
                    prev = None
                    for m in range(MT):
                        if q == 0 and m == 0:
                            aw2s = aw2s0
                        else:
                            aw2s = issue_act_mul(m, ncol0, scs)
                        if m + 1 < MT:
                            scs = issue_scores(m + 1, ncol0)
                        # AV issued one m-tile late: its multiply finished a
                        # full phase ago, so the matmul's sem wait is
                        # pre-satisfied and the tensor stream never pauses
                        if prev is not None:
                            issue_av(prev[0], prev[1], oa)
                        prev = (m, aw2s)
                        if pending and m >= 1:
                            pending.pop(0)()
                    issue_av(prev[0], prev[1], oa)
                    pending = make_norm_steps(q, oa)

                # q3 normalize flush (inside the oa pool scope)
                for step in pending:
                    step()
                pending = []

          # output projection: all attention PSUM pools closed, banks free
          with tc.tile_pool(name="prj", bufs=4, space="PSUM") as prj, \
               tc.tile_pool(name="otp", bufs=4) as otp:
            for nt in range(MT):
                pp = prj.tile([128, OUP], f32)
                nc.tensor.matmul(
                    pp[:],
                    lhsT=aoutT[:, nt * 128:(nt + 1) * 128],
                    rhs=woutc[:],
                    start=True, stop=True,
                )
                ot = otp.tile([128, OUP], f32)
                nc.scalar.copy(out=ot[:], in_=pp[:])
                nc.gpsimd.dma_start(
                    out=out_d[nt * 128:(nt + 1) * 128, :], in_=ot[:]
                )
    nc.compile()
    return nc


_NC_CACHE = {}


def kernel(x, w_qkv, bias_table, w_out, b_out, relative_pos):
    x = np.asarray(x, np.float32)
    w_qkv = np.asarray(w_qkv, np.float32)
    bias_table = np.asarray(bias_table, np.float32)
    w_out = np.asarray(w_out, np.float32)
    b_out = np.asarray(b_out, np.float32)
    relative_pos = np.asarray(relative_pos, np.int32)

    bias = bias_table[relative_pos, 0]                       # [n, m]
    expBT = np.exp(bias.T - C_SHIFT).astype(np.float16)      # [m, n]
    expBT = np.ascontiguousarray(expBT)

    if "nc" not in _NC_CACHE:
        _NC_CACHE["nc"] = _build_nc()
    nc = _NC_CACHE["nc"]

    in_maps = []
    for c in range(8):
        b, g = c // 2, c % 2
        w_qk = np.concatenate(
            [w_qkv[:, g * 128:(g + 1) * 128],
             w_qkv[:, 256 + g * 128:256 + (g + 1) * 128]], axis=1)
        in_maps.append({
            "xT": np.ascontiguousarray(x[b].T).reshape(2, 128, N).astype(np.float16),
            "w_qk": np.ascontiguousarray(w_qk).reshape(2, 128, 256).astype(np.float16),
            "w_v": np.ascontiguousarray(
                w_qkv[:, 512 + g * 128:512 + (g + 1) * 128]
            ).reshape(2, 128, 128).astype(np.float16),
            "w_out2": np.ascontiguousarray(
                w_out[g * 128:(g + 1) * 128, :]
            ).astype(np.float16),
            "expbt": expBT,
        })

    trace = bool(os.environ.get("KERNEL_TRACE"))
    res = run_bass_kernel_spmd(nc, in_maps, list(range(8)), trace=trace)
    _LAST["exec_time_ns"] = res.exec_time_ns
    _LAST["results"] = res

    parts = [np.asarray(res.results[c]["partial"], np.float32) for c in range(8)]
    out = np.stack([parts[2 * b] + parts[2 * b + 1] + b_out for b in range(B)])
    return out.astype(np.float32)
